# revision 1
# baseline (speedup 1.0000x reference)
"""Trainium2 Bass kernel for GraphTransitionModel (GNN message passing).

Model (per batch element b, N=256 nodes):
  x[i]   = (obs[b,i], i/N)                              node features, 2-dim
  h1     = relu(W0a^T x_i + W0b^T x_j + a*w4 + b0)      messenger layer 1, 64
  h2     = relu(W1^T h1 + b1)                           64
  h3     = relu(W2^T h2 + b2)                           64
  m(i,j) = w3 . h3 + b3                                 scalar
  msg[i] = sum_j m(i,j) = w3 . (sum_j h3) + N*b3
  u      = MLP_updater([x_i, msg[i]])  (3->64->64->64->1)
  out[b,i] = u

Strategy: pure data parallel, 4 batch elements per core x 8 cores.
On-chip layout: features on partitions, pairs on the free dim.
Two i-rows (i and i+128) are stacked into 128 partitions; the 64x64
layers run as 128x128 block-diagonal matmuls.  The final w3-dot plus
sum over j is folded into a per-i free-dim accumulation (accum_out).

Sync-wait budget: Trainium matmul (S3_LW) carries a single sync-wait
slot, so every matmul's operands must be reachable through one
semaphore: constants arrive via ONE packed DMA, a barrier + dummy PE
matmul absorbs that dep, and multi-writer tiles (qb, pb, s2) are
fenced through single DVE copies.
"""

import os
import sys
import numpy as np

sys.path.insert(0, "/opt/trn_rl_repo")

B, N, MID = 32, 256, 64
NCORES = 8
BPC = B // NCORES  # batches per core = 4
HALF = N // 2  # 128 stacked tiles per batch

# wpack column layout
C_W1BD = 0
C_W2BD = 128
C_UW1 = 256
C_UW2 = 320
C_W0A = 384
C_W0B = 448
C_UW0 = 512
C_W3S = 576
C_B1S = 578
C_B2S = 579
C_UW3 = 580
C_UB0 = 581
C_UB1 = 582
C_UB2 = 583
C_SCAL = 584
C_TOT = 586


def _build_bass():
    import concourse.bass as bass
    import concourse.bacc as bacc
    import concourse.tile as tile
    from concourse import mybir

    f32 = mybir.dt.float32
    AF = mybir.ActivationFunctionType
    ALU = mybir.AluOpType

    nc = bacc.Bacc("TRN2", target_bir_lowering=False, num_devices=NCORES)

    wpack_d = nc.declare_dram_parameter("wpack", [128, C_TOT], f32, isOutput=False)
    xT_d = nc.declare_dram_parameter("xT", [BPC, 2, N], f32, isOutput=False)
    ab0_d = nc.declare_dram_parameter("ab0", [BPC, MID, 1], f32, isOutput=False)
    out_d = nc.declare_dram_parameter("out", [BPC, N], f32, isOutput=True)

    with tile.TileContext(nc) as tc:
        with (
            tc.tile_pool(name="consts", bufs=1) as consts,
            tc.tile_pool(name="perb", bufs=2) as perb,
            tc.tile_pool(name="work", bufs=3) as work,
            tc.tile_pool(name="ps_main", bufs=3, space="PSUM") as ps_main,
            tc.tile_pool(name="ps_main2", bufs=3, space="PSUM") as ps_main2,
            tc.tile_pool(name="ps_small", bufs=1, space="PSUM") as ps_small,
            tc.tile_pool(name="ps_warm", bufs=1, space="PSUM") as ps_warm,
        ):
            wp = consts.tile([128, C_TOT], f32, tag="wpack")
            nc.sync.dma_start(out=wp[:], in_=wpack_d[:])
            w1bd = wp[:, C_W1BD : C_W1BD + 128]
            w2bd = wp[:, C_W2BD : C_W2BD + 128]
            uw1 = wp[0:MID, C_UW1 : C_UW1 + MID]
            uw2 = wp[0:MID, C_UW2 : C_UW2 + MID]
            w0a = wp[0:2, C_W0A : C_W0A + MID]
            w0b = wp[0:2, C_W0B : C_W0B + MID]
            uw0 = wp[0:3, C_UW0 : C_UW0 + MID]
            w3s = wp[:, C_W3S : C_W3S + 2]
            b1s = wp[:, C_B1S : C_B1S + 1]
            b2s = wp[:, C_B2S : C_B2S + 1]
            uw3 = wp[0:MID, C_UW3 : C_UW3 + 1]
            ub0 = wp[0:MID, C_UB0 : C_UB0 + 1]
            ub1 = wp[0:MID, C_UB1 : C_UB1 + 1]
            ub2 = wp[0:MID, C_UB2 : C_UB2 + 1]
            scal = wp[:, C_SCAL : C_SCAL + 2]

            # Dummy PE matmul absorbs the wpack-DMA wait so later matmuls
            # (single sync-wait slot) only wait on their RAW producer engine.
            psw = ps_warm.tile([1, 1], f32, tag="warm")
            nc.tensor.matmul(psw[:], w1bd[:, 0:1], w1bd[:, 0:1], start=True, stop=True)

            for b in range(BPC):
                # ---- per-batch setup ----
                uin = perb.tile([3, N], f32, tag="uin")
                nc.sync.dma_start(out=uin[0:2, :], in_=xT_d[b])
                ab0s = perb.tile([128, 1], f32, tag="ab0s")
                src = ab0_d[b]
                ab0_bcast = bass.AP(
                    tensor=src.tensor,
                    offset=src.offset,
                    ap=[[0, 2]] + list(src.ap),
                )
                nc.sync.dma_start(out=ab0s[:], in_=ab0_bcast)

                psP = ps_small.tile([MID, N], f32, tag="pss")
                nc.tensor.matmul(psP[:], w0a, uin[0:2, :], start=True, stop=True)
                p1 = perb.tile([MID, N], f32, tag="p1")
                nc.scalar.copy(p1[:], psP[:])

                psQ = ps_small.tile([MID, N], f32, tag="pss")
                nc.tensor.matmul(psQ[:], w0b, uin[0:2, :], start=True, stop=True)
                qb = perb.tile([128, N], f32, tag="qb")
                nc.scalar.activation(qb[0:MID, :], psQ[:], AF.Identity, bias=ab0s[0:MID, :])
                nc.sync.dma_start(out=qb[MID:128, :], in_=qb[0:MID, :])

                pb = perb.tile([128, HALF], f32, tag="pb")
                nc.sync.dma_start(out=pb[0:MID, :], in_=p1[:, 0:HALF])
                nc.sync.dma_start(out=pb[MID:128, :], in_=p1[:, HALF:N])

                # DVE fences: h1's tensor_scalar then depends only on DVE
                if b == 0:
                    zeros = consts.tile([128, N], f32, tag="zeros")
                    nc.vector.memset(zeros[:], 0.0)
                qb2 = perb.tile([128, N], f32, tag="qb2")
                nc.vector.tensor_copy(qb2[:], qb[:])
                pb2 = perb.tile([128, HALF], f32, tag="pb2")
                nc.vector.tensor_copy(pb2[:], pb[:])

                # S2[c, t] accumulates sum_j h3 for i=t (upper) / i=t+HALF (lower)
                # ACT and DVE accumulate into separate tiles to avoid any
                # false cross-engine WAW serialization on a shared tile
                s2 = perb.tile([128, HALF], f32, tag="s2")
                s2a = perb.tile([128, (HALF + 2) // 3], f32, tag="s2a")

                # ---- main pair loop ----
                for t in range(HALF):
                    h1 = work.tile([128, N], f32, tag="h1")
                    nc.vector.tensor_scalar(
                        h1[:], qb2[:], pb2[:, t : t + 1], 0.0, ALU.add, ALU.max
                    )
                    ps1 = ps_main.tile([128, N], f32, tag="ps1")
                    nc.tensor.matmul(ps1[:], w1bd, h1[:], start=True, stop=True)
                    h2 = work.tile([128, N], f32, tag="h2")
                    nc.scalar.activation(h2[:], ps1[:], AF.Relu, bias=b1s)
                    ps2 = ps_main2.tile([128, N], f32, tag="ps2")
                    nc.tensor.matmul(ps2[:], w2bd, h2[:], start=True, stop=True)
                    h3 = work.tile([128, N], f32, tag="h3")
                    if t % 3 != 0:
                        # relu+sum-accum on DVE: (ps2 + b2s) max 0; accum is
                        # hardwired to sum for scalar_tensor_tensor
                        nc.vector.scalar_tensor_tensor(
                            h3[:], ps2[:], b2s, zeros[:], ALU.add, ALU.max,
                            accum_out=s2[:, t : t + 1],
                        )
                    else:
                        nc.scalar.activation(
                            h3[:], ps2[:], AF.Relu, bias=b2s,
                            accum_out=s2a[:, t // 3 : t // 3 + 1],
                        )

                # ---- msg = w3s^T @ S2  -> [2, HALF] ----
                s2f = perb.tile([128, HALF], f32, tag="s2f")
                nc.vector.tensor_copy(s2f[:], s2[:])
                sel = bass.AP(tensor=s2f.tensor, offset=s2f.offset,
                              ap=[s2f.ap[0], [3, (HALF + 2) // 3]])
                nc.vector.tensor_copy(sel, s2a[:])
                psm = ps_small.tile([2, HALF], f32, tag="pss")
                nc.tensor.matmul(psm[:], w3s, s2f[:], start=True, stop=True)
                msg2 = perb.tile([2, HALF], f32, tag="msg2")
                nc.scalar.activation(msg2[:], psm[:], AF.Identity, bias=scal[0:2, 0:1])
                # flatten [2, HALF] -> row 2 of uin [1, N]
                nc.sync.dma_start(out=uin[2:3, :], in_=msg2[:])

                # ---- updater MLP ----
                psu1 = ps_small.tile([MID, N], f32, tag="pss")
                nc.tensor.matmul(psu1[:], uw0, uin[:], start=True, stop=True)
                t1 = perb.tile([MID, N], f32, tag="t1")
                nc.scalar.activation(t1[:], psu1[:], AF.Relu, bias=ub0)
                psu2 = ps_small.tile([MID, N], f32, tag="pss")
                nc.tensor.matmul(psu2[:], uw1, t1[:], start=True, stop=True)
                t2 = perb.tile([MID, N], f32, tag="t2")
                nc.scalar.activation(t2[:], psu2[:], AF.Relu, bias=ub1)
                psu3 = ps_small.tile([MID, N], f32, tag="pss")
                nc.tensor.matmul(psu3[:], uw2, t2[:], start=True, stop=True)
                t3 = perb.tile([MID, N], f32, tag="t3")
                nc.scalar.activation(t3[:], psu3[:], AF.Relu, bias=ub2)
                pso = ps_small.tile([1, N], f32, tag="pss")
                nc.tensor.matmul(pso[:], uw3, t3[:], start=True, stop=True)
                orow = perb.tile([1, N], f32, tag="orow")
                nc.scalar.activation(orow[:], pso[:], AF.Identity, bias=scal[0:1, 1:2])
                nc.sync.dma_start(out=out_d[b], in_=orow[:])

    nc.compile()
    return nc


def _host_inputs(inputs):
    g = lambda k: np.asarray(inputs[k], np.float32)
    obs, action = g("obs"), g("action")
    m_w0, m_b0, m_w1, m_b1 = g("m_w0"), g("m_b0"), g("m_w1"), g("m_b1")
    m_w2, m_b2, m_w3, m_b3 = g("m_w2"), g("m_b2"), g("m_w3"), g("m_b3")
    u_w0, u_b0, u_w1, u_b1 = g("u_w0"), g("u_b0"), g("u_w1"), g("u_b1")
    u_w2, u_b2, u_w3, u_b3 = g("u_w2"), g("u_b2"), g("u_w3"), g("u_b3")

    coor = np.arange(N, dtype=np.float32) / N
    xT = np.stack([obs, np.broadcast_to(coor, obs.shape)], axis=1)  # [B, 2, N]
    ab0 = (action[:, None] * m_w0[4] + m_b0).astype(np.float32)[..., None]

    wpack = np.zeros((128, C_TOT), np.float32)
    wpack[:MID, C_W1BD : C_W1BD + MID] = m_w1
    wpack[MID:, C_W1BD + MID : C_W1BD + 128] = m_w1
    wpack[:MID, C_W2BD : C_W2BD + MID] = m_w2
    wpack[MID:, C_W2BD + MID : C_W2BD + 128] = m_w2
    wpack[:MID, C_UW1 : C_UW1 + MID] = u_w1
    wpack[:MID, C_UW2 : C_UW2 + MID] = u_w2
    wpack[0:2, C_W0A : C_W0A + MID] = m_w0[0:2]
    wpack[0:2, C_W0B : C_W0B + MID] = m_w0[2:4]
    wpack[0:3, C_UW0 : C_UW0 + MID] = u_w0
    wpack[:MID, C_W3S] = m_w3[:, 0]
    wpack[MID:, C_W3S + 1] = m_w3[:, 0]
    wpack[:MID, C_B1S] = m_b1
    wpack[MID:, C_B1S] = m_b1
    wpack[:MID, C_B2S] = m_b2
    wpack[MID:, C_B2S] = m_b2
    wpack[:MID, C_UW3] = u_w3[:, 0]
    wpack[:MID, C_UB0] = u_b0
    wpack[:MID, C_UB1] = u_b1
    wpack[:MID, C_UB2] = u_b2
    wpack[0:2, C_SCAL] = N * float(m_b3[0])
    wpack[0:2, C_SCAL + 1] = float(u_b3[0])

    in_maps = []
    for c in range(NCORES):
        sl = slice(c * BPC, (c + 1) * BPC)
        in_maps.append(
            dict(
                wpack=wpack,
                xT=np.ascontiguousarray(xT[sl]),
                ab0=np.ascontiguousarray(ab0[sl]),
            )
        )
    return in_maps


def kernel(**inputs) -> np.ndarray:
    in_maps = _host_inputs(inputs)

    from concourse.bass_utils import run_bass_kernel_spmd

    nc = _build_bass()
    res = run_bass_kernel_spmd(
        nc, in_maps, core_ids=list(range(NCORES)),
        trace=bool(int(os.environ.get("KERNEL_TRACE", "0"))),
    )
    out = np.concatenate([r["out"] for r in res.results], axis=0)  # [B, N]
    if res.exec_time_ns is not None:
        print(f"HW exec time: {res.exec_time_ns} ns")
        print(f"mean exec time: {res.mean_exec_time_ns} ns")
    return out.astype(np.float32)


if __name__ == "__main__":
    nc = _build_bass()
    print("bass build OK")



# revision 8
# speedup vs baseline: 1.3659x; 1.3659x over previous
"""Trainium2 Bass kernel for GraphTransitionModel (GNN message passing).

Model (per batch element b, N=256 nodes):
  x[i]   = (obs[b,i], i/N)                              node features, 2-dim
  h1     = relu(A^T x_i + B^T x_j + a*w4 + b0)          messenger layer 1, 64
  h2     = relu(W1^T h1 + b1)                           64
  h3     = relu(W2^T h2 + b2)                           64
  m(i,j) = w3 . h3 + b3                                 scalar
  msg[i] = sum_j m(i,j)
  out    = MLP_updater([x_i, msg[i]])  (3->64->64->64->1)

Strategy: pure data parallel, 4 batch elements per core x 8 cores.

Device layout ("j-loop"): iterate over j; free dim carries all 256 i's.
Two j-rows (j and j+128) are stacked into 128 partitions; the 64x64
layers run as 128x128 block-diagonal bf16 matmuls (1 cycle/col on PE
vs 4 for fp32).  The sum over j is folded into PSUM accumulation of a
tiny third matmul (w3s stationary, start=False across the j-loop), so
no accumulator drains or free-dim reductions are needed.

Elementwise balance per 4-j chunk (free dim 1024):
  DVE: 4x h1 tensor_scalar (bf16 4x mode) + h3 cols [0:HS)
  ACT: h2 relu over [128,1024] + h3 cols [HS:1024)
mm3 reads h3 in 256-col pieces so each matmul sees a single-writer
tile.  The updater MLP runs once at the end over all 4 batches
(free dim 1024, layers 2-4 in bf16).

Sync-wait discipline (single sync-wait slot on matmul): constants via
packed DMAs absorbed by dummy PE matmuls; multi-writer tiles (Pd, Qs)
fenced through single DVE copies before the pair loop reads them.
"""

import os
import sys
import numpy as np

sys.path.insert(0, "/opt/trn_rl_repo")

B, N, MID = 32, 256, 64
NCORES = 8
BPC = B // NCORES  # batches per core = 4
HALF = N // 2  # 128 stacked j-iterations per batch
JCH = 4  # j's per chunk
NCH = HALF // JCH  # 32 chunks per batch
FREE = JCH * N  # 1024 free columns per chunk
HS = 768  # h3 column split: [0:HS) on DVE, [HS:FREE) on ACT

# fp32 wpack column layout
C_W0A = 0
C_W0B = 64
C_UW0 = 128
C_B1S = 192
C_B2S = 193
C_UB0 = 194
C_UB1 = 195
C_UB2 = 196
C_UB3 = 197
C_UW1 = 198
C_UW2 = 262
C_UW3 = 326
C_TOT32 = 327

# bf16 wpack16 column layout
C_W1BD = 0
C_W2BD = 128
C_W3S = 256
C_TOT16 = 258


def _build_bass():
    import concourse.bass as bass
    import concourse.bacc as bacc
    import concourse.tile as tile
    from concourse import mybir

    f32 = mybir.dt.float32
    bf16 = mybir.dt.bfloat16
    AF = mybir.ActivationFunctionType
    ALU = mybir.AluOpType

    nc = bacc.Bacc("TRN2", target_bir_lowering=False, num_devices=NCORES)

    wp_d = nc.declare_dram_parameter("wpack", [128, C_TOT32], f32, isOutput=False)
    wp16_d = nc.declare_dram_parameter("wpack16", [128, C_TOT16], bf16, isOutput=False)
    xT_d = nc.declare_dram_parameter("xT", [BPC, 2, N], f32, isOutput=False)
    ab0_d = nc.declare_dram_parameter("ab0", [BPC, MID, 1], f32, isOutput=False)
    out_d = nc.declare_dram_parameter("out", [BPC, N], f32, isOutput=True)

    with tile.TileContext(nc) as tc:
        with (
            tc.tile_pool(name="consts", bufs=1) as consts,
            tc.tile_pool(name="perb", bufs=2) as perb,
            tc.tile_pool(name="wk1", bufs=4) as wk1,
            tc.tile_pool(name="wk2", bufs=2) as wk2,
            tc.tile_pool(name="wk3a", bufs=2) as wk3a,
            tc.tile_pool(name="wk3b", bufs=2) as wk3b,
            tc.tile_pool(name="wkt", bufs=3) as wkt,
            tc.tile_pool(name="ps_z2", bufs=2, space="PSUM") as ps_z2,
            tc.tile_pool(name="ps_z3", bufs=1, space="PSUM") as ps_z3,
            tc.tile_pool(name="ps_msg", bufs=1, space="PSUM") as ps_msg,
            tc.tile_pool(name="ps_small", bufs=1, space="PSUM") as ps_small,
        ):
            wp = consts.tile([128, C_TOT32], f32, tag="wpack")
            nc.sync.dma_start(out=wp[:], in_=wp_d[:])
            wp16 = consts.tile([128, C_TOT16], bf16, tag="wpack16")
            nc.sync.dma_start(out=wp16[:], in_=wp16_d[:])

            w0a = wp[0:2, C_W0A : C_W0A + MID]
            w0b = wp[0:2, C_W0B : C_W0B + MID]
            uw0 = wp[0:4, C_UW0 : C_UW0 + MID]
            b1s = wp[:, C_B1S : C_B1S + 1]
            b2s = wp[:, C_B2S : C_B2S + 1]
            ub0 = wp[0:MID, C_UB0 : C_UB0 + 1]
            ub1 = wp[0:MID, C_UB1 : C_UB1 + 1]
            ub2 = wp[0:MID, C_UB2 : C_UB2 + 1]
            ub3 = wp[0:1, C_UB3 : C_UB3 + 1]

            w1bd = wp16[:, C_W1BD : C_W1BD + 128]
            w2bd = wp16[:, C_W2BD : C_W2BD + 128]
            w3s = wp16[:, C_W3S : C_W3S + 2]
            uw1 = wp[0:MID, C_UW1 : C_UW1 + MID]
            uw2 = wp[0:MID, C_UW2 : C_UW2 + MID]
            uw3 = wp[0:MID, C_UW3 : C_UW3 + 1]

            # Dummy PE matmuls absorb the two const-DMA waits so later
            # matmuls (single sync-wait slot) only wait on their RAW
            # producer engine.
            psw = ps_small.tile([1, 1], f32, tag="pss")
            nc.tensor.matmul(psw[:], wp[0:1, 0:1], wp[0:1, 0:1], start=True, stop=True)
            psw2 = ps_small.tile([1, 1], f32, tag="pss")
            nc.tensor.matmul(
                psw2[:], wp16[0:1, 0:1], wp16[0:1, 0:1], start=True, stop=True
            )

            # updater input for all batches: rows [obs, coor, msgA, msgB]
            uin = consts.tile([4, BPC * N], f32, tag="uin")

            for b in range(BPC):
                bc = slice(b * N, (b + 1) * N)
                # ---- per-batch setup ----
                nc.sync.dma_start(out=uin[0:2, bc], in_=xT_d[b])
                ab0s = perb.tile([MID, 1], f32, tag="ab0s")
                nc.sync.dma_start(out=ab0s[:], in_=ab0_d[b])

                psP = ps_small.tile([MID, N], f32, tag="pss")
                nc.tensor.matmul(psP[:], w0a, uin[0:2, bc], start=True, stop=True)
                Pd = perb.tile([128, N], bf16, tag="Pd")
                nc.scalar.copy(Pd[0:MID, :], psP[:])
                nc.sync.dma_start(out=Pd[MID:128, :], in_=Pd[0:MID, :])

                psQ = ps_small.tile([MID, N], f32, tag="pss")
                nc.tensor.matmul(psQ[:], w0b, uin[0:2, bc], start=True, stop=True)
                qtmp = perb.tile([MID, N], f32, tag="qtmp")
                nc.scalar.activation(qtmp[:], psQ[:], AF.Identity, bias=ab0s)
                Qs = perb.tile([128, HALF], f32, tag="Qs")
                nc.sync.dma_start(out=Qs[0:MID, :], in_=qtmp[:, 0:HALF])
                nc.sync.dma_start(out=Qs[MID:128, :], in_=qtmp[:, HALF:N])

                # DVE fences: single-writer sources for the pair loop
                Pd2 = perb.tile([128, N], bf16, tag="Pd2")
                nc.vector.tensor_copy(Pd2[:], Pd[:])
                Qs2 = perb.tile([128, HALF], f32, tag="Qs2")
                nc.vector.tensor_copy(Qs2[:], Qs[:])

                # msg accumulator for this batch: two j-half sums
                msg_ps = ps_msg.tile([2, N], f32, tag="msg")

                # ---- j-loop, 1-chunk software pipeline ----
                zprev = None
                for c in range(NCH + 1):
                    if c < NCH:
                        jb = c * JCH
                        h1a = wk1.tile([128, 2 * N], bf16, tag="h1a")
                        nc.vector.tensor_scalar(
                            h1a[:, 0:N], Pd2[:], Qs2[:, jb : jb + 1],
                            0.0, ALU.add, ALU.max,
                        )
                        nc.vector.tensor_scalar(
                            h1a[:, N : 2 * N], Pd2[:], Qs2[:, jb + 1 : jb + 2],
                            0.0, ALU.add, ALU.max,
                        )
                        h1b = wk1.tile([128, 2 * N], bf16, tag="h1b")
                        nc.vector.tensor_scalar(
                            h1b[:, 0:N], Pd2[:], Qs2[:, jb + 2 : jb + 3],
                            0.0, ALU.add, ALU.max,
                        )
                        nc.vector.tensor_scalar(
                            h1b[:, N : 2 * N], Pd2[:], Qs2[:, jb + 3 : jb + 4],
                            0.0, ALU.add, ALU.max,
                        )
                        z2 = ps_z2.tile([128, FREE], f32, tag="z2")
                        nc.tensor.matmul(
                            z2[:, 0 : 2 * N], w1bd, h1a[:], start=True, stop=True
                        )
                        nc.tensor.matmul(
                            z2[:, 2 * N : FREE], w1bd, h1b[:], start=True, stop=True
                        )
                    if zprev is not None:
                        h3a = wk3a.tile([128, HS], bf16, tag="h3a")
                        nc.vector.tensor_scalar(
                            h3a[:], zprev[:, 0:HS], b2s, 0.0, ALU.add, ALU.max
                        )
                        h3b = wk3b.tile([128, FREE - HS], bf16, tag="h3b")
                        nc.scalar.activation(
                            h3b[:], zprev[:, HS:FREE], AF.Relu, bias=b2s
                        )
                    if c < NCH:
                        h2 = wk2.tile([128, FREE], bf16, tag="h2")
                        nc.scalar.activation(h2[:], z2[:], AF.Relu, bias=b1s)
                    if zprev is not None:
                        for q in range(JCH):
                            lo = q * N
                            if lo + N <= HS:
                                src = h3a[:, lo : lo + N]
                            else:
                                src = h3b[:, lo - HS : lo - HS + N]
                            first = (c == 1) and (q == 0)
                            last = (c == NCH) and (q == JCH - 1)
                            nc.tensor.matmul(
                                msg_ps[:], w3s, src,
                                start=first, stop=last, skip_group_check=True,
                            )
                    if c < NCH:
                        z3 = ps_z3.tile([128, FREE], f32, tag="z3")
                        nc.tensor.matmul(
                            z3[:, 0 : 2 * N], w2bd, h2[:, 0 : 2 * N],
                            start=True, stop=True,
                        )
                        nc.tensor.matmul(
                            z3[:, 2 * N : FREE], w2bd, h2[:, 2 * N : FREE],
                            start=True, stop=True,
                        )
                        zprev = z3
                    else:
                        zprev = None

                # msg PSUM -> SBUF -> uin rows 2:4 (DMA crosses partitions)
                msg_sb = perb.tile([2, N], f32, tag="msg_sb")
                nc.scalar.copy(msg_sb[:], msg_ps[:])
                nc.sync.dma_start(out=uin[2:4, bc], in_=msg_sb[:])

            # ---- updater MLP, all batches at once (free = BPC*N) ----
            FB = BPC * N
            psu1 = ps_z2.tile([MID, FB], f32, tag="z2")
            nc.tensor.matmul(
                psu1[:, 0 : FB // 2], uw0, uin[:, 0 : FB // 2], start=True, stop=True
            )
            nc.tensor.matmul(
                psu1[:, FB // 2 : FB], uw0, uin[:, FB // 2 : FB], start=True, stop=True
            )
            t1 = wkt.tile([MID, FB], f32, tag="t1")
            nc.scalar.activation(t1[:], psu1[:], AF.Relu, bias=ub0)
            psu2 = ps_z3.tile([MID, FB], f32, tag="z3")
            nc.tensor.matmul(
                psu2[:, 0 : FB // 2], uw1, t1[:, 0 : FB // 2], start=True, stop=True
            )
            nc.tensor.matmul(
                psu2[:, FB // 2 : FB], uw1, t1[:, FB // 2 : FB], start=True, stop=True
            )
            t2 = wkt.tile([MID, FB], f32, tag="t2")
            nc.scalar.activation(t2[:], psu2[:], AF.Relu, bias=ub1)
            psu3 = ps_z2.tile([MID, FB], f32, tag="z2")
            nc.tensor.matmul(
                psu3[:, 0 : FB // 2], uw2, t2[:, 0 : FB // 2], start=True, stop=True
            )
            nc.tensor.matmul(
                psu3[:, FB // 2 : FB], uw2, t2[:, FB // 2 : FB], start=True, stop=True
            )
            t3 = wkt.tile([MID, FB], f32, tag="t3")
            nc.scalar.activation(t3[:], psu3[:], AF.Relu, bias=ub2)
            pso = ps_z3.tile([1, FB], f32, tag="z3")
            nc.tensor.matmul(
                pso[:, 0 : FB // 2], uw3, t3[:, 0 : FB // 2], start=True, stop=True
            )
            nc.tensor.matmul(
                pso[:, FB // 2 : FB], uw3, t3[:, FB // 2 : FB], start=True, stop=True
            )
            orow = consts.tile([1, FB], f32, tag="orow")
            nc.scalar.activation(orow[:], pso[:], AF.Identity, bias=ub3)
            for b in range(BPC):
                nc.sync.dma_start(
                    out=out_d[b], in_=orow[0:1, b * N : (b + 1) * N]
                )

    nc.compile()
    return nc


def _host_inputs(inputs):
    import ml_dtypes

    g = lambda k: np.asarray(inputs[k], np.float32)
    obs, action = g("obs"), g("action")
    m_w0, m_b0, m_w1, m_b1 = g("m_w0"), g("m_b0"), g("m_w1"), g("m_b1")
    m_w2, m_b2, m_w3, m_b3 = g("m_w2"), g("m_b2"), g("m_w3"), g("m_b3")
    u_w0, u_b0, u_w1, u_b1 = g("u_w0"), g("u_b0"), g("u_w1"), g("u_b1")
    u_w2, u_b2, u_w3, u_b3 = g("u_w2"), g("u_b2"), g("u_w3"), g("u_b3")

    coor = np.arange(N, dtype=np.float32) / N
    xT = np.stack([obs, np.broadcast_to(coor, obs.shape)], axis=1)  # [B, 2, N]
    # per-batch action/bias for Q: a*w4 + b0
    ab0 = (action[:, None] * m_w0[4] + m_b0).astype(np.float32)[..., None]

    wpack = np.zeros((128, C_TOT32), np.float32)
    wpack[0:2, C_W0A : C_W0A + MID] = m_w0[0:2]
    wpack[0:2, C_W0B : C_W0B + MID] = m_w0[2:4]
    wpack[0:2, C_UW0 : C_UW0 + MID] = u_w0[0:2]
    wpack[2, C_UW0 : C_UW0 + MID] = u_w0[2]
    wpack[3, C_UW0 : C_UW0 + MID] = u_w0[2]
    wpack[:MID, C_B1S] = m_b1
    wpack[MID:, C_B1S] = m_b1
    wpack[:MID, C_B2S] = m_b2
    wpack[MID:, C_B2S] = m_b2
    # msg rows exclude the +N*b3 term; fold it into the updater bias
    wpack[:MID, C_UB0] = u_b0 + N * float(m_b3[0]) * u_w0[2]
    wpack[:MID, C_UB1] = u_b1
    wpack[:MID, C_UB2] = u_b2
    wpack[0, C_UB3] = float(u_b3[0])
    wpack[:MID, C_UW1 : C_UW1 + MID] = u_w1
    wpack[:MID, C_UW2 : C_UW2 + MID] = u_w2
    wpack[:MID, C_UW3] = u_w3[:, 0]

    wpack16 = np.zeros((128, C_TOT16), np.float32)
    wpack16[:MID, C_W1BD : C_W1BD + MID] = m_w1
    wpack16[MID:, C_W1BD + MID : C_W1BD + 128] = m_w1
    wpack16[:MID, C_W2BD : C_W2BD + MID] = m_w2
    wpack16[MID:, C_W2BD + MID : C_W2BD + 128] = m_w2
    wpack16[:MID, C_W3S] = m_w3[:, 0]
    wpack16[MID:, C_W3S + 1] = m_w3[:, 0]
    wpack16 = wpack16.astype(ml_dtypes.bfloat16)

    in_maps = []
    for c in range(NCORES):
        sl = slice(c * BPC, (c + 1) * BPC)
        in_maps.append(
            dict(
                wpack=wpack,
                wpack16=wpack16,
                xT=np.ascontiguousarray(xT[sl]),
                ab0=np.ascontiguousarray(ab0[sl]),
            )
        )
    return in_maps


def kernel(**inputs) -> np.ndarray:
    in_maps = _host_inputs(inputs)

    from concourse.bass_utils import run_bass_kernel_spmd

    nc = _build_bass()
    res = run_bass_kernel_spmd(
        nc, in_maps, core_ids=list(range(NCORES)),
        trace=bool(int(os.environ.get("KERNEL_TRACE", "0"))),
    )
    out = np.concatenate([r["out"] for r in res.results], axis=0)  # [B, N]
    if res.exec_time_ns is not None:
        print(f"HW exec time: {res.exec_time_ns} ns")
        print(f"mean exec time: {res.mean_exec_time_ns} ns")
    return out.astype(np.float32)


if __name__ == "__main__":
    nc = _build_bass()
    print("bass build OK")
